# revision 40
# baseline (speedup 1.0000x reference)
"""GCN encoder (5-layer GCNConv + global mean pool) on 8 Trainium2 NeuronCores.

Strategy (node sharding, v2):
  - 10000 nodes split contiguously across 8 cores (1250/core, padded 1280).
  - Edges (incl. self-loops) bucketed by (dst core, dst tile of 128,
    src half); src half = first/second 640 local rows of the src's core.
    One shared Seg (GCN norm folded into one-hot values) + gather-index
    table drives the aggregation matmuls of ALL 5 layers. Buckets are
    ragged (per-bucket chunk counts); idx slots beyond the max-core valid
    count are -1 (SWDGE skips them).
  - Layer 1 aggregates x directly from replicated fp32 x tables
    (ExternalInput, half-major layout) - no collective. Messages are cast
    fp16 on the Activation engine before the one-hot matmuls.
  - Layers 2-5: per-tile GEMM (h @ W, fp16, PSUM fp32) -> fp16 cast (ACT)
    -> per-half bounce -> per-half AllGather into a 5120-row shared table
    -> SWDGE gathers (prepare_only + trigger_dma, 2 queues, pipelined) ->
    one-hot matmul aggregation. The h0 AllGather overlaps the previous
    layer's tail; the h1 AllGather overlaps the h0 aggregation pass.
    Bias folds into the h0 partial copy (DVE); relu+cast runs on ACT;
    transposes on PE.
  - Mean-pool as matmul with 1/count one-hot, AllReduce over cores.

Graph structure (edge sort, Seg with norm, gather indices, pool matrix)
is preprocessed on host; all FLOPs on x/W run on device.
"""
import sys

import numpy as np

sys.path.insert(0, "/opt/trn_rl_repo")

import concourse.bacc as bacc
import concourse.bass as bass  # noqa: F401
import concourse.mybir as mybir
import concourse.tile as tile
from concourse import bass_utils
from concourse.instruction_name_ordered_set import InstructionNameOrderedSet

dt = mybir.dt
AF = mybir.ActivationFunctionType

N = 10000
E = 150000
G = 64
C = 8
DIN = 128
DHID = 512
DOUT = 128
NPC = N // C          # 1250 nodes per core
NTILE = 10            # tiles of 128 dst nodes per core
TSPLIT = 3            # src-half split: tiles 0-2 | 3-9 (asymmetric so the
                      # next layer's h0 AllGather can fire early)
SPLIT = TSPLIT * 128          # 384 local rows in half 0
H1PC = 896                    # padded half-1 rows per core (866 real)
HR = [C * SPLIT, C * H1PC]    # rows per half table [3072, 7168]
FO = [DHID, DHID, DHID, DHID, DOUT]   # per-layer GEMM output widths
FIT = [1, 4, 4, 4, 4]                 # fi tiles per GEMM


def _preprocess(edge_index, batch):
    """Bucket edges by (src half, dst tile) per dst core; build Seg, gather
    indices (half-local rows, -1 padded), and the pool matrix."""
    src = np.concatenate([edge_index[0].astype(np.int64), np.arange(N, dtype=np.int64)])
    dst = np.concatenate([edge_index[1].astype(np.int64), np.arange(N, dtype=np.int64)])
    deg = np.bincount(dst, minlength=N).astype(np.float64)
    dinv = np.where(deg > 0, 1.0 / np.sqrt(deg), 0.0)
    norm = (dinv[src] * dinv[dst]).astype(np.float32)

    core = dst // NPC
    t_of = (dst % NPC) // 128
    dloc = (dst % NPC) % 128

    s_core = src // NPC
    s_loc = src % NPC
    s_half = (s_loc >= SPLIT).astype(np.int64)
    s_row = np.where(s_half == 0, s_core * SPLIT + s_loc,
                     s_core * H1PC + (s_loc - SPLIT))   # row within half table

    # bucket key, h-major: (src half, dst tile) within each dst core.
    # Duplicate src rows within a bucket share one gather slot (the Seg
    # column for that slot is multi-hot with summed norms).
    hb = s_half * NTILE + t_of          # 0..19
    order = np.lexsort((s_row, hb, core))
    b_row, b_hb, b_core, b_dloc, b_norm = (
        s_row[order], hb[order], core[order], dloc[order], norm[order])
    gbucket = b_core * (2 * NTILE) + b_hb
    new_grp = np.ones(len(order), bool)
    new_grp[1:] = (gbucket[1:] != gbucket[:-1]) | (b_row[1:] != b_row[:-1])
    uid = np.cumsum(new_grp) - 1
    bnd = np.ones(len(order), bool)
    bnd[1:] = gbucket[1:] != gbucket[:-1]
    start_uid = np.maximum.accumulate(np.where(bnd, uid, -1))
    slot = uid - start_uid

    ucnt = np.zeros((C, 2 * NTILE), np.int64)
    np.add.at(ucnt, (b_core, b_hb), new_grp)
    mreg = ucnt.max(axis=0)             # uniform valid slot count per bucket
    kp = np.maximum((mreg + 127) // 128, 1)   # chunks per bucket
    base = np.zeros(2 * NTILE, np.int64)
    base[1:] = np.cumsum(kp)[:-1]
    nchunk = int(kp.sum())

    chunk = base[b_hb] + slot // 128
    erow = slot % 128

    gidx = np.full((C, nchunk * 128), -1, np.int16)
    seg32 = np.zeros((C, 128, nchunk, 128), np.float32)
    gidx[b_core, chunk * 128 + erow] = b_row.astype(np.int16)
    np.add.at(seg32, (b_core, erow, chunk, b_dloc), b_norm)
    seg = seg32.astype(np.float16)
    # pad every bucket to the uniform valid count with idx-0 fillers
    for c in range(C):
        for b in range(2 * NTILE):
            lo, hi = base[b] * 128 + ucnt[c, b], base[b] * 128 + mreg[b]
            gidx[c, lo:hi] = 0

    # idx wrap: logical idx i -> partition i%16, column i//16; replicate x8
    gidx_w = np.ascontiguousarray(
        np.tile(gidx.reshape(C, -1, 16).transpose(0, 2, 1), (1, 8, 1)))

    # pool matrix [C, 128, NTILE, G]: 1/count at (node row, graph)
    gcnt = np.bincount(batch, minlength=G).astype(np.float64)
    inv = (1.0 / np.maximum(gcnt, 1.0))
    pool = np.zeros((C, 128, NTILE, G), np.float16)
    nodes = np.arange(N)
    pc, pr = nodes // NPC, nodes % NPC
    pool[pc, pr % 128, pr // 128, batch] = inv[batch].astype(np.float16)

    meta = (tuple(int(v) for v in kp), tuple(int(v) for v in base),
            tuple(int(v) for v in mreg), nchunk)
    return gidx_w, seg, pool, meta


def _xtables(x):
    """Half-major replicated x tables, fp32."""
    h0 = np.zeros((HR[0], DIN), np.float32)
    h1 = np.zeros((HR[1], DIN), np.float32)
    for c in range(C):
        h0[c * SPLIT:(c + 1) * SPLIT] = x[c * NPC: c * NPC + SPLIT]
        h1[c * H1PC: c * H1PC + NPC - SPLIT] = x[c * NPC + SPLIT:(c + 1) * NPC]
    return h0, h1


def _build(meta):
    kp, cbase, mreg, nchunk = meta
    kpmax = max(kp)
    nc = bacc.Bacc("TRN2", target_bir_lowering=False, debug=False,
                   num_devices=C, num_swdge_queues=4)

    xt_in = [nc.dram_tensor(f"xt{h}_in", [HR[h], DIN], dt.float32,
                            kind="ExternalInput") for h in range(2)]
    w_in = [nc.dram_tensor(f"w{i}_in", [DIN if i == 0 else DHID, FO[i]],
                           dt.float32, kind="ExternalInput") for i in range(5)]
    b_in = [nc.dram_tensor(f"b{i}_in", [128, FO[i]], dt.float32,
                           kind="ExternalInput") for i in range(5)]
    seg_in = nc.dram_tensor("seg_in", [128, nchunk, 128], dt.float16,
                            kind="ExternalInput")
    gidx_in = nc.dram_tensor("gidx_in", [128, nchunk * 8], dt.int16,
                             kind="ExternalInput")
    pool_in = nc.dram_tensor("pool_in", [128, NTILE, G], dt.float16,
                             kind="ExternalInput")
    id_in = nc.dram_tensor("id_in", [128, 128], dt.float16, kind="ExternalInput")
    out = nc.dram_tensor("out", [G, DOUT], dt.float32, kind="ExternalOutput")

    hpc = [SPLIT, H1PC]
    gshA = [nc.dram_tensor(f"gshA{h}", [HR[h], DHID], dt.float16,
                           addr_space="Shared") for h in range(2)]
    gshB = [nc.dram_tensor(f"gshB{h}", [HR[h], DOUT], dt.float16,
                           addr_space="Shared") for h in range(2)]
    bounceA = [nc.dram_tensor(f"bounceA{h}", [hpc[h], DHID], dt.float16)
               for h in range(2)]
    bounceB = [nc.dram_tensor(f"bounceB{h}", [hpc[h], DOUT], dt.float16)
               for h in range(2)]
    pool_sh = nc.dram_tensor("pool_sh", [G, DOUT], dt.float32, addr_space="Shared")
    pool_bounce = nc.dram_tensor("pool_bounce", [G, DOUT], dt.float32)

    mset_count = {}

    with tile.TileContext(nc) as tc:
        with (
            tc.tile_pool(name="const", bufs=1) as cp,
            tc.tile_pool(name="work", bufs=2) as wp,
            tc.tile_pool(name="msgpA", bufs=5) as mpA,
            tc.tile_pool(name="msgp", bufs=3) as mp,
            tc.tile_pool(name="gemm_ps", bufs=2, space="PSUM") as gps,
            tc.tile_pool(name="agg_ps", bufs=2, space="PSUM") as aps,
            tc.tile_pool(name="tp_ps", bufs=2, space="PSUM") as tps,
            tc.tile_pool(name="pool_ps", bufs=1, space="PSUM") as pps,
        ):
            # ---- resident tensors (gidx first: it gates the first gather) ----
            gidx_sb = cp.tile([128, nchunk * 8], dt.int16)
            nc.sync.dma_start(out=gidx_sb[:, :], in_=gidx_in[:, :])
            seg_sb = cp.tile([128, nchunk, 128], dt.float16)
            c10 = cbase[NTILE]       # first h1 chunk: split the load so the
            nc.sync.dma_start(       # h0 matmuls aren't gated on the full 11MB
                out=seg_sb[:, :c10, :], in_=seg_in[:, :c10, :])
            nc.sync.dma_start(
                out=seg_sb[:, c10:, :], in_=seg_in[:, c10:, :])
            pool_sb = cp.tile([128, NTILE, G], dt.float16)
            id16 = cp.tile([128, 128], dt.float16)
            nc.sync.dma_start(out=id16[:, :], in_=id_in[:, :])
            breps = cp.tile([128, 4, DHID], dt.float32)
            for l in range(2):
                nc.sync.dma_start(out=breps[:, l, :], in_=b_in[l][:, :])
            brep5 = cp.tile([128, DOUT], dt.float32)

            # weights -> fp16 tiles. slots: W1 -> w16[:,0]; W2..W4 -> 1+4(i-1)+j
            w16 = cp.tile([128, 13, DHID], dt.float16)
            w516 = cp.tile([128, 4, DOUT], dt.float16)

            def load_weight(i):
                for j in range(FIT[i]):
                    wstage = wp.tile([128, FO[i]], dt.float32, tag="wstage")
                    nc.sync.dma_start(
                        out=wstage[:, :], in_=w_in[i][j * 128:(j + 1) * 128, :])
                    if i < 4:
                        nc.vector.tensor_copy(
                            w16[:, (0 if i == 0 else 1 + 4 * (i - 1)) + j, :],
                            wstage[:, :])
                    else:
                        nc.vector.tensor_copy(w516[:, j, :], wstage[:, :])

            # W1/W2 are needed during L1's POSTs; the rest are deferred into
            # layer 2's AllGather gap to keep L1's DMA window for gathers
            load_weight(0)
            load_weight(1)

            hT = cp.tile([128, NTILE, 4, 128], dt.float16)
            h_out = cp.tile([128, NTILE, DOUT], dt.float16)
            partial = cp.tile([128, NTILE, DHID], dt.float16)
            pp = pps.tile([64, DOUT], dt.float32)

            def gather_bucket(l, h, t, bi):
                """Prep+trigger the SWDGE gather for bucket (h,t)."""
                q = (2 * bi) % 4
                b = h * NTILE + t
                if l == 1:
                    mtile = mp.tile([128, kpmax, DIN], dt.float32, tag="m32")
                    src, esz = xt_in[h], DIN
                elif l == 5:
                    mtile = mp.tile([128, kpmax, DOUT], dt.float16, tag="mB")
                    src, esz = gshB[h], DOUT
                else:
                    mtile = mpA.tile([128, kpmax, DHID], dt.float16, tag="mA")
                    src, esz = gshA[h], DHID
                # zero the skipped -1 tail slots: the gather leaves them
                # unwritten, and NaN garbage there would poison the Seg-0
                # matmul columns (0*NaN=NaN)
                kc = mreg[b] // 128
                if kc < kp[b]:
                    nc.vector.memset(
                        mtile[:, kc:kp[b], :].rearrange("p a b -> p (a b)"),
                        0.0)
                # split the bucket across both SWDGE queues so the two
                # transfers overlap (one queue's ring serializes batches)
                ka = (kp[b] + 1) // 2
                for s, (k0, k1) in enumerate(((0, ka), (ka, kp[b]))):
                    if k1 <= k0:
                        continue
                    nreg = min(mreg[b], k1 * 128) - min(mreg[b], k0 * 128)
                    if nreg <= 0:
                        continue
                    prep = nc.gpsimd.dma_gather(
                        out_ap=mtile[:, k0:k1, :],
                        in_ap=src[:, :],
                        idxs_ap=gidx_sb[:, (cbase[b] + k0) * 8:
                                        (cbase[b] + k1) * 8],
                        num_idxs=(k1 - k0) * 128,
                        num_idxs_reg=nreg,
                        elem_size=esz,
                        single_packet=False,
                        queue_num=(q + s) % 4,
                    )
                    # chain gathers with a no-sync edge: the scheduler must
                    # not swap them, or the round-robin DMASW-lane/queue
                    # pairing breaks (lane sems are locked to one queue)
                    if gather_bucket.last is not None:
                        deps = InstructionNameOrderedSet()
                        deps.add(gather_bucket.last)
                        prep.ins.add_nosync_dependencies_from(deps)
                    gather_bucket.last = prep.ins.name
                return mtile

            def agg_bucket(l, h, t, mtile):
                """One-hot matmul accumulation of bucket (h,t) into a fresh
                PSUM aggregator; returns the aggregator tile."""
                b = h * NTILE + t
                fo = DIN if l == 1 else FO[l - 1]
                if l == 1:
                    m16 = mp.tile([128, kpmax, DIN], dt.float16, tag="m16")
                    nc.scalar.activation(
                        m16[:, :kp[b], :].rearrange("p a b -> p (a b)"),
                        mtile[:, :kp[b], :].rearrange("p a b -> p (a b)"),
                        AF.Copy)
                    mtile = m16
                pa = aps.tile([128, DHID], dt.float32, tag="pa")
                for k in range(kp[b]):
                    nc.tensor.matmul(
                        pa[:, :fo], seg_sb[:, cbase[b] + k, :], mtile[:, k, :],
                        start=(k == 0), stop=(k == kp[b] - 1))
                return pa

            def post_tile(l, t):
                """After both halves of tile t are aggregated for layer l:
                finish the tile and stage the next layer's table."""
                fo = DIN if l == 1 else FO[l - 1]
                hsum = wp.tile([128, fo], dt.float32, tag=f"hsum{fo}")
                nc.vector.tensor_tensor(
                    hsum[:, :], post_tile.pa[:, :fo], partial[:, t, :fo],
                    mybir.AluOpType.add)
                if l == 1:
                    # (Ax) -> fp16 -> transpose -> @W1 + b1 -> relu
                    st16 = wp.tile([128, DIN], dt.float16, tag="st16")
                    nc.scalar.activation(st16[:, :], hsum[:, :], AF.Copy)
                    pt1 = tps.tile([128, DHID], dt.float16, tag="pt")
                    nc.tensor.transpose(pt1[:, :128], st16[:, :], id16[:, :])
                    xT = wp.tile([128, DIN], dt.float16, tag="xT")
                    nc.vector.tensor_copy(xT[:, :], pt1[:, :128])
                    pg = gps.tile([128, DHID], dt.float32, tag="pg")
                    nc.tensor.matmul(pg[:, :], xT[:, :], w16[:, 0, :],
                                     start=True, stop=True)
                    hs2 = wp.tile([128, DHID], dt.float32, tag="hsum512")
                    nc.vector.tensor_tensor(
                        hs2[:, :], pg[:, :], breps[:, 0, :],
                        mybir.AluOpType.add)
                    hnm = wp.tile([128, DHID], dt.float16, tag="hnm")
                    nc.scalar.activation(hnm[:, :], hs2[:, :], AF.Relu)
                elif l < 5:
                    hnm = wp.tile([128, fo], dt.float16, tag="hnm")
                    nc.scalar.activation(hnm[:, :], hsum[:, :], AF.Relu)
                else:
                    nc.scalar.activation(h_out[:, t, :], hsum[:, :], AF.Relu)
                    nc.tensor.matmul(
                        pp[:, :], pool_sb[:, t, :64], h_out[:, t, :],
                        start=(t == 0), stop=(t == NTILE - 1))
                    return

                # transposes -> hT -> GEMM W_{l+1} -> cast -> bounce half
                fon = FO[l]
                bounce = bounceA if l < 4 else bounceB
                pt = tps.tile([128, DHID], dt.float16, tag="pt")
                for j in range(4):
                    nc.tensor.transpose(
                        pt[:, j * 128:(j + 1) * 128],
                        hnm[:, j * 128:(j + 1) * 128], id16[:, :])
                nc.vector.tensor_copy(
                    hT[:, t, :, :].rearrange("p a b -> p (a b)"), pt[:, :512])
                pg2 = gps.tile([128, fon], dt.float32, tag="pg")
                for j in range(4):
                    wslot = (w16[:, 1 + 4 * (l - 1) + j, :] if l < 4
                             else w516[:, j, :])
                    nc.tensor.matmul(pg2[:, :], hT[:, t, j, :], wslot,
                                     start=(j == 0), stop=(j == 3))
                hw16 = wp.tile([128, fon], dt.float16, tag="hw16")
                nc.scalar.activation(hw16[:, :], pg2[:, :], AF.Copy)
                hh = 0 if t < TSPLIT else 1
                r = t * 128 if t < TSPLIT else (t - TSPLIT) * 128
                nc.sync.dma_start(out=bounce[hh].ap()[r:r + 128, :],
                                  in_=hw16[:, :])

            def ag_half(bounce, gsh):
                nc.gpsimd.collective_compute(
                    "AllGather", mybir.AluOpType.bypass,
                    replica_groups=[list(range(C))],
                    ins=[bounce.ap().opt()],
                    outs=[gsh.ap().opt()])

            gather_bucket.last = None

            # ========================= LAYERS 1..5 =========================
            # Pool-stream order per layer: [AG(l,h0) trigger] h0 gathers,
            # [AG(l,h1) trigger] h1 gathers. The AG triggers wait on the
            # previous layer's bounce writes; placing them at half-pass
            # heads keeps them from head-of-line-blocking gather issue.
            # post_tile lags the aggregation by one bucket so its
            # DVE->ACT->PE chain hides under the next bucket's matmuls.
            bi = 0
            for l in range(1, 6):
                fo = DIN if l == 1 else FO[l - 1]
                # deferred const loads ride this layer's AllGather gap
                # (W_{l+1}/b_{l} are first needed during layer l's passes)
                if l == 2:
                    load_weight(2)
                    nc.sync.dma_start(out=breps[:, 2, :], in_=b_in[2][:, :])
                elif l == 3:
                    load_weight(3)
                    nc.sync.dma_start(out=breps[:, 3, :], in_=b_in[3][:, :])
                elif l == 4:
                    load_weight(4)
                    nc.sync.dma_start(out=brep5[:, :], in_=b_in[4][:, :])
                    nc.sync.dma_start(out=pool_sb[:, :, :], in_=pool_in[:, :, :])
                for h in range(2):
                    pend = None
                    for t in range(NTILE):
                        mt = gather_bucket(l, h, t, bi); bi += 1
                        if l < 5 and h == 1 and t == 5:
                            # trigger the next layer's h0 table AllGather
                            # from this layer's gather stream once
                            # POST(l,t0..2)'s bounces land
                            ag_half((bounceA if l < 4 else bounceB)[0],
                                    (gshA if l < 4 else gshB)[0])
                        pa = agg_bucket(l, h, t, mt)
                        if h == 0:
                            # fold the post-agg bias into the partial copy
                            if l == 1:
                                nc.vector.tensor_copy(
                                    partial[:, t, :fo], pa[:, :fo])
                            else:
                                nc.vector.tensor_tensor(
                                    partial[:, t, :fo], pa[:, :fo],
                                    breps[:, l - 1, :fo] if l < 5
                                    else brep5[:, :],
                                    mybir.AluOpType.add)
                        else:
                            if pend is not None:
                                post_tile.pa = pend[1]
                                post_tile(l, pend[0])
                            pend = (t, pa)
                    if h == 1:
                        post_tile.pa = pend[1]
                        post_tile(l, pend[0])
                        if l < 5:
                            # h1 table AllGather: emitted after POST(l,t9)
                            # exists; hides under AG(h0) + the h0 gather pass
                            ag_half((bounceA if l < 4 else bounceB)[1],
                                    (gshA if l < 4 else gshB)[1])

            # ---- mean pool: AllReduce over cores ----
            pres = wp.tile([64, DOUT], dt.float32, tag="pres")
            nc.vector.tensor_copy(pres[:, :], pp[:, :])
            nc.sync.dma_start(out=pool_bounce[:, :], in_=pres[:, :])
            nc.gpsimd.collective_compute(
                "AllReduce", mybir.AluOpType.add,
                replica_groups=[list(range(C))],
                ins=[pool_bounce.ap().opt()],
                outs=[pool_sh.ap().opt()])
            ores = wp.tile([64, DOUT], dt.float32, tag="ores")
            nc.sync.dma_start(out=ores[:, :], in_=pool_sh[:, :])
            nc.sync.dma_start(out=out[:, :], in_=ores[:, :])

    nc.compile()
    return nc


_CACHE = {}


def _get_program(meta):
    if meta not in _CACHE:
        _CACHE[meta] = _build(meta)
    return _CACHE[meta]


def make_in_maps(inputs):
    edge_index = np.asarray(inputs["edge_index"])
    batch = np.asarray(inputs["batch"])
    x = np.asarray(inputs["x"], dtype=np.float32)
    gidx_w, seg, pool, meta = _preprocess(edge_index, batch)
    xt0, xt1 = _xtables(x)
    ident = np.eye(128, dtype=np.float16)
    in_maps = []
    for c in range(C):
        m = {
            "xt0_in": xt0,
            "xt1_in": xt1,
            "seg_in": np.ascontiguousarray(seg[c]),
            "gidx_in": gidx_w[c],
            "pool_in": np.ascontiguousarray(pool[c]),
            "id_in": ident,
        }
        for i in range(5):
            w = np.asarray(inputs[f"W{i + 1}"], dtype=np.float32)
            b = np.asarray(inputs[f"b{i + 1}"], dtype=np.float32)
            m[f"w{i}_in"] = w
            m[f"b{i}_in"] = np.ascontiguousarray(np.tile(b[None, :], (128, 1)))
        in_maps.append(m)
    return in_maps, meta


def kernel(**inputs):
    in_maps, meta = make_in_maps(inputs)
    nc = _get_program(meta)
    res = bass_utils.run_bass_kernel_spmd(
        nc, in_maps, core_ids=list(range(C)))
    return res.results[0]["out"].astype(np.float32)


# revision 41
# speedup vs baseline: 1.1191x; 1.1191x over previous
"""GCN encoder (5-layer GCNConv + global mean pool) on 8 Trainium2 NeuronCores.

Strategy (node sharding, v2):
  - 10000 nodes split contiguously across 8 cores (1250/core, padded 1280).
  - Edges (incl. self-loops) bucketed by (dst core, dst tile of 128,
    src half); src half = first/second 640 local rows of the src's core.
    One shared Seg (GCN norm folded into one-hot values) + gather-index
    table drives the aggregation matmuls of ALL 5 layers. Buckets are
    ragged (per-bucket chunk counts); idx slots beyond the max-core valid
    count are -1 (SWDGE skips them).
  - Layer 1 aggregates x directly from replicated fp32 x tables
    (ExternalInput, half-major layout) - no collective. Messages are cast
    fp16 on the Activation engine before the one-hot matmuls.
  - Layers 2-5: per-tile GEMM (h @ W, fp16, PSUM fp32) -> fp16 cast (ACT)
    -> per-half bounce -> per-half AllGather into a 5120-row shared table
    -> SWDGE gathers (prepare_only + trigger_dma, 2 queues, pipelined) ->
    one-hot matmul aggregation. The h0 AllGather overlaps the previous
    layer's tail; the h1 AllGather overlaps the h0 aggregation pass.
    Bias folds into the h0 partial copy (DVE); relu+cast runs on ACT;
    transposes on PE.
  - Mean-pool as matmul with 1/count one-hot, AllReduce over cores.

Graph structure (edge sort, Seg with norm, gather indices, pool matrix)
is preprocessed on host; all FLOPs on x/W run on device.
"""
import sys

import numpy as np

sys.path.insert(0, "/opt/trn_rl_repo")

import concourse.bacc as bacc
import concourse.bass as bass  # noqa: F401
import concourse.mybir as mybir
import concourse.tile as tile
from concourse import bass_utils

dt = mybir.dt
AF = mybir.ActivationFunctionType

N = 10000
E = 150000
G = 64
C = 8
DIN = 128
DHID = 512
DOUT = 128
NPC = N // C          # 1250 nodes per core
NTILE = 10            # tiles of 128 dst nodes per core
HROWS = C * 640       # 5120 rows per half table
FO = [DHID, DHID, DHID, DHID, DOUT]   # per-layer GEMM output widths
FIT = [1, 4, 4, 4, 4]                 # fi tiles per GEMM


def _preprocess(edge_index, batch):
    """Bucket edges by (src half, dst tile) per dst core; build Seg, gather
    indices (half-local rows, -1 padded), and the pool matrix."""
    src = np.concatenate([edge_index[0].astype(np.int64), np.arange(N, dtype=np.int64)])
    dst = np.concatenate([edge_index[1].astype(np.int64), np.arange(N, dtype=np.int64)])
    deg = np.bincount(dst, minlength=N).astype(np.float64)
    dinv = np.where(deg > 0, 1.0 / np.sqrt(deg), 0.0)
    norm = (dinv[src] * dinv[dst]).astype(np.float32)

    core = dst // NPC
    t_of = (dst % NPC) // 128
    dloc = (dst % NPC) % 128

    s_core = src // NPC
    s_loc = src % NPC
    s_half = (s_loc >= 640).astype(np.int64)
    s_row = s_core * 640 + (s_loc - s_half * 640)   # row within half table

    # bucket key, h-major: (src half, dst tile) within each dst core.
    # Duplicate src rows within a bucket share one gather slot (the Seg
    # column for that slot is multi-hot with summed norms).
    hb = s_half * NTILE + t_of          # 0..19
    order = np.lexsort((s_row, hb, core))
    b_row, b_hb, b_core, b_dloc, b_norm = (
        s_row[order], hb[order], core[order], dloc[order], norm[order])
    gbucket = b_core * (2 * NTILE) + b_hb
    new_grp = np.ones(len(order), bool)
    new_grp[1:] = (gbucket[1:] != gbucket[:-1]) | (b_row[1:] != b_row[:-1])
    uid = np.cumsum(new_grp) - 1
    bnd = np.ones(len(order), bool)
    bnd[1:] = gbucket[1:] != gbucket[:-1]
    start_uid = np.maximum.accumulate(np.where(bnd, uid, -1))
    slot = uid - start_uid

    ucnt = np.zeros((C, 2 * NTILE), np.int64)
    np.add.at(ucnt, (b_core, b_hb), new_grp)
    mreg = ucnt.max(axis=0)             # uniform valid slot count per bucket
    kp = np.maximum((mreg + 127) // 128, 1)   # chunks per bucket
    base = np.zeros(2 * NTILE, np.int64)
    base[1:] = np.cumsum(kp)[:-1]
    nchunk = int(kp.sum())

    chunk = base[b_hb] + slot // 128
    erow = slot % 128

    gidx = np.full((C, nchunk * 128), -1, np.int16)
    seg32 = np.zeros((C, 128, nchunk, 128), np.float32)
    gidx[b_core, chunk * 128 + erow] = b_row.astype(np.int16)
    np.add.at(seg32, (b_core, erow, chunk, b_dloc), b_norm)
    seg = seg32.astype(np.float16)
    # pad every bucket to the uniform valid count with idx-0 fillers
    for c in range(C):
        for b in range(2 * NTILE):
            lo, hi = base[b] * 128 + ucnt[c, b], base[b] * 128 + mreg[b]
            gidx[c, lo:hi] = 0

    # idx wrap: logical idx i -> partition i%16, column i//16; replicate x8
    gidx_w = np.ascontiguousarray(
        np.tile(gidx.reshape(C, -1, 16).transpose(0, 2, 1), (1, 8, 1)))

    # pool matrix [C, 128, NTILE, G]: 1/count at (node row, graph)
    gcnt = np.bincount(batch, minlength=G).astype(np.float64)
    inv = (1.0 / np.maximum(gcnt, 1.0))
    pool = np.zeros((C, 128, NTILE, G), np.float16)
    nodes = np.arange(N)
    pc, pr = nodes // NPC, nodes % NPC
    pool[pc, pr % 128, pr // 128, batch] = inv[batch].astype(np.float16)

    meta = (tuple(int(v) for v in kp), tuple(int(v) for v in base),
            tuple(int(v) for v in mreg), nchunk)
    return gidx_w, seg, pool, meta


def _xtables(x):
    """Half-major replicated x tables [5120, 128] fp32 each."""
    h0 = np.zeros((HROWS, DIN), np.float32)
    h1 = np.zeros((HROWS, DIN), np.float32)
    for c in range(C):
        h0[c * 640:(c + 1) * 640] = x[c * NPC: c * NPC + 640]
        h1[c * 640: c * 640 + NPC - 640] = x[c * NPC + 640:(c + 1) * NPC]
    return h0, h1


def _build(meta):
    kp, cbase, mreg, nchunk = meta
    kpmax = max(kp)
    nc = bacc.Bacc("TRN2", target_bir_lowering=False, debug=False,
                   num_devices=C, num_swdge_queues=4)

    xt_in = [nc.dram_tensor(f"xt{h}_in", [HROWS, DIN], dt.float32,
                            kind="ExternalInput") for h in range(2)]
    w_in = [nc.dram_tensor(f"w{i}_in", [DIN if i == 0 else DHID, FO[i]],
                           dt.float32, kind="ExternalInput") for i in range(5)]
    b_in = [nc.dram_tensor(f"b{i}_in", [128, FO[i]], dt.float32,
                           kind="ExternalInput") for i in range(5)]
    seg_in = nc.dram_tensor("seg_in", [128, nchunk, 128], dt.float16,
                            kind="ExternalInput")
    gidx_in = nc.dram_tensor("gidx_in", [128, nchunk * 8], dt.int16,
                             kind="ExternalInput")
    pool_in = nc.dram_tensor("pool_in", [128, NTILE, G], dt.float16,
                             kind="ExternalInput")
    id_in = nc.dram_tensor("id_in", [128, 128], dt.float16, kind="ExternalInput")
    out = nc.dram_tensor("out", [G, DOUT], dt.float32, kind="ExternalOutput")

    gshA = [nc.dram_tensor(f"gshA{h}", [HROWS, DHID], dt.float16,
                           addr_space="Shared") for h in range(2)]
    gshB = [nc.dram_tensor(f"gshB{h}", [HROWS, DOUT], dt.float16,
                           addr_space="Shared") for h in range(2)]
    bounceA = [nc.dram_tensor(f"bounceA{h}", [640, DHID], dt.float16)
               for h in range(2)]
    bounceB = [nc.dram_tensor(f"bounceB{h}", [640, DOUT], dt.float16)
               for h in range(2)]
    pool_sh = nc.dram_tensor("pool_sh", [G, DOUT], dt.float32, addr_space="Shared")
    pool_bounce = nc.dram_tensor("pool_bounce", [G, DOUT], dt.float32)

    mset_count = {}

    with tile.TileContext(nc) as tc:
        with (
            tc.tile_pool(name="const", bufs=1) as cp,
            tc.tile_pool(name="work", bufs=2) as wp,
            tc.tile_pool(name="msgp", bufs=4) as mp,
            tc.tile_pool(name="gemm_ps", bufs=2, space="PSUM") as gps,
            tc.tile_pool(name="agg_ps", bufs=2, space="PSUM") as aps,
            tc.tile_pool(name="tp_ps", bufs=2, space="PSUM") as tps,
            tc.tile_pool(name="pool_ps", bufs=1, space="PSUM") as pps,
        ):
            # ---- resident tensors (gidx first: it gates the first gather) ----
            gidx_sb = cp.tile([128, nchunk * 8], dt.int16)
            nc.sync.dma_start(out=gidx_sb[:, :], in_=gidx_in[:, :])
            seg_sb = cp.tile([128, nchunk, 128], dt.float16)
            c10 = cbase[NTILE]       # first h1 chunk: split the load so the
            nc.sync.dma_start(       # h0 matmuls aren't gated on the full 11MB
                out=seg_sb[:, :c10, :], in_=seg_in[:, :c10, :])
            nc.sync.dma_start(
                out=seg_sb[:, c10:, :], in_=seg_in[:, c10:, :])
            pool_sb = cp.tile([128, NTILE, G], dt.float16)
            id16 = cp.tile([128, 128], dt.float16)
            nc.sync.dma_start(out=id16[:, :], in_=id_in[:, :])
            breps = cp.tile([128, 4, DHID], dt.float32)
            for l in range(2):
                nc.sync.dma_start(out=breps[:, l, :], in_=b_in[l][:, :])
            brep5 = cp.tile([128, DOUT], dt.float32)

            # weights -> fp16 tiles. slots: W1 -> w16[:,0]; W2..W4 -> 1+4(i-1)+j
            w16 = cp.tile([128, 13, DHID], dt.float16)
            w516 = cp.tile([128, 4, DOUT], dt.float16)

            def load_weight(i):
                for j in range(FIT[i]):
                    wstage = wp.tile([128, FO[i]], dt.float32, tag="wstage")
                    nc.sync.dma_start(
                        out=wstage[:, :], in_=w_in[i][j * 128:(j + 1) * 128, :])
                    if i < 4:
                        nc.vector.tensor_copy(
                            w16[:, (0 if i == 0 else 1 + 4 * (i - 1)) + j, :],
                            wstage[:, :])
                    else:
                        nc.vector.tensor_copy(w516[:, j, :], wstage[:, :])

            # W1/W2 are needed during L1's POSTs; the rest are deferred into
            # layer 2's AllGather gap to keep L1's DMA window for gathers
            load_weight(0)
            load_weight(1)

            hT = cp.tile([128, NTILE, 4, 128], dt.float16)
            h_out = cp.tile([128, NTILE, DOUT], dt.float16)
            partial = cp.tile([128, NTILE, DHID], dt.float16)
            pp = pps.tile([64, DOUT], dt.float32)

            def gather_bucket(l, h, t, bi):
                """Prep+trigger the SWDGE gather for bucket (h,t)."""
                q = (2 * bi) % 4
                b = h * NTILE + t
                if l == 1:
                    mtile = mp.tile([128, kpmax, DIN], dt.float32, tag="m32")
                    src, esz = xt_in[h], DIN
                elif l == 5:
                    mtile = mp.tile([128, kpmax, DOUT], dt.float16, tag="mB")
                    src, esz = gshB[h], DOUT
                else:
                    mtile = mp.tile([128, kpmax, DHID], dt.float16, tag="mA")
                    src, esz = gshA[h], DHID
                # zero the skipped -1 tail slots: the gather leaves them
                # unwritten, and NaN garbage there would poison the Seg-0
                # matmul columns (0*NaN=NaN)
                kc = mreg[b] // 128
                if kc < kp[b]:
                    nc.vector.memset(
                        mtile[:, kc:kp[b], :].rearrange("p a b -> p (a b)"),
                        0.0)
                # split the bucket across both SWDGE queues so the two
                # transfers overlap (one queue's ring serializes batches)
                ka = (kp[b] + 1) // 2
                for s, (k0, k1) in enumerate(((0, ka), (ka, kp[b]))):
                    if k1 <= k0:
                        continue
                    nreg = min(mreg[b], k1 * 128) - min(mreg[b], k0 * 128)
                    if nreg <= 0:
                        continue
                    nc.gpsimd.dma_gather(
                        out_ap=mtile[:, k0:k1, :],
                        in_ap=src[:, :],
                        idxs_ap=gidx_sb[:, (cbase[b] + k0) * 8:
                                        (cbase[b] + k1) * 8],
                        num_idxs=(k1 - k0) * 128,
                        num_idxs_reg=nreg,
                        elem_size=esz,
                        single_packet=False,
                        queue_num=(q + s) % 4,
                    )
                return mtile

            def agg_bucket(l, h, t, mtile):
                """One-hot matmul accumulation of bucket (h,t) into a fresh
                PSUM aggregator; returns the aggregator tile."""
                b = h * NTILE + t
                fo = DIN if l == 1 else FO[l - 1]
                if l == 1:
                    m16 = mp.tile([128, kpmax, DIN], dt.float16, tag="m16")
                    nc.scalar.activation(
                        m16[:, :kp[b], :].rearrange("p a b -> p (a b)"),
                        mtile[:, :kp[b], :].rearrange("p a b -> p (a b)"),
                        AF.Copy)
                    mtile = m16
                pa = aps.tile([128, DHID], dt.float32, tag="pa")
                for k in range(kp[b]):
                    nc.tensor.matmul(
                        pa[:, :fo], seg_sb[:, cbase[b] + k, :], mtile[:, k, :],
                        start=(k == 0), stop=(k == kp[b] - 1))
                return pa

            def post_tile(l, t):
                """After both halves of tile t are aggregated for layer l:
                finish the tile and stage the next layer's table."""
                fo = DIN if l == 1 else FO[l - 1]
                hsum = wp.tile([128, fo], dt.float32, tag=f"hsum{fo}")
                nc.vector.tensor_tensor(
                    hsum[:, :], post_tile.pa[:, :fo], partial[:, t, :fo],
                    mybir.AluOpType.add)
                if l == 1:
                    # (Ax) -> fp16 -> transpose -> @W1 + b1 -> relu
                    st16 = wp.tile([128, DIN], dt.float16, tag="st16")
                    nc.scalar.activation(st16[:, :], hsum[:, :], AF.Copy)
                    pt1 = tps.tile([128, DHID], dt.float16, tag="pt")
                    nc.tensor.transpose(pt1[:, :128], st16[:, :], id16[:, :])
                    xT = wp.tile([128, DIN], dt.float16, tag="xT")
                    nc.vector.tensor_copy(xT[:, :], pt1[:, :128])
                    pg = gps.tile([128, DHID], dt.float32, tag="pg")
                    nc.tensor.matmul(pg[:, :], xT[:, :], w16[:, 0, :],
                                     start=True, stop=True)
                    hs2 = wp.tile([128, DHID], dt.float32, tag="hsum512")
                    nc.vector.tensor_tensor(
                        hs2[:, :], pg[:, :], breps[:, 0, :],
                        mybir.AluOpType.add)
                    hnm = wp.tile([128, DHID], dt.float16, tag="hnm")
                    nc.scalar.activation(hnm[:, :], hs2[:, :], AF.Relu)
                elif l < 5:
                    hnm = wp.tile([128, fo], dt.float16, tag="hnm")
                    nc.scalar.activation(hnm[:, :], hsum[:, :], AF.Relu)
                else:
                    nc.scalar.activation(h_out[:, t, :], hsum[:, :], AF.Relu)
                    nc.tensor.matmul(
                        pp[:, :], pool_sb[:, t, :64], h_out[:, t, :],
                        start=(t == 0), stop=(t == NTILE - 1))
                    return

                # transposes -> hT -> GEMM W_{l+1} -> cast -> bounce half
                fon = FO[l]
                bounce = bounceA if l < 4 else bounceB
                pt = tps.tile([128, DHID], dt.float16, tag="pt")
                for j in range(4):
                    nc.tensor.transpose(
                        pt[:, j * 128:(j + 1) * 128],
                        hnm[:, j * 128:(j + 1) * 128], id16[:, :])
                nc.vector.tensor_copy(
                    hT[:, t, :, :].rearrange("p a b -> p (a b)"), pt[:, :512])
                pg2 = gps.tile([128, fon], dt.float32, tag="pg")
                for j in range(4):
                    wslot = (w16[:, 1 + 4 * (l - 1) + j, :] if l < 4
                             else w516[:, j, :])
                    nc.tensor.matmul(pg2[:, :], hT[:, t, j, :], wslot,
                                     start=(j == 0), stop=(j == 3))
                hw16 = wp.tile([128, fon], dt.float16, tag="hw16")
                nc.scalar.activation(hw16[:, :], pg2[:, :], AF.Copy)
                hh, r = t // 5, (t % 5) * 128
                nc.sync.dma_start(out=bounce[hh].ap()[r:r + 128, :],
                                  in_=hw16[:, :])

            def ag_half(bounce, gsh):
                nc.gpsimd.collective_compute(
                    "AllGather", mybir.AluOpType.bypass,
                    replica_groups=[list(range(C))],
                    ins=[bounce.ap().opt()],
                    outs=[gsh.ap().opt()])

            # ========================= LAYERS 1..5 =========================
            # Pool-stream order per layer: [AG(l,h0) trigger] h0 gathers,
            # [AG(l,h1) trigger] h1 gathers. The AG triggers wait on the
            # previous layer's bounce writes; placing them at half-pass
            # heads keeps them from head-of-line-blocking gather issue.
            # post_tile lags the aggregation by one bucket so its
            # DVE->ACT->PE chain hides under the next bucket's matmuls.
            bi = 0
            for l in range(1, 6):
                fo = DIN if l == 1 else FO[l - 1]
                # deferred const loads ride this layer's AllGather gap
                # (W_{l+1}/b_{l} are first needed during layer l's passes)
                if l == 2:
                    load_weight(2)
                    nc.sync.dma_start(out=breps[:, 2, :], in_=b_in[2][:, :])
                elif l == 3:
                    load_weight(3)
                    nc.sync.dma_start(out=breps[:, 3, :], in_=b_in[3][:, :])
                elif l == 4:
                    load_weight(4)
                    nc.sync.dma_start(out=brep5[:, :], in_=b_in[4][:, :])
                    nc.sync.dma_start(out=pool_sb[:, :, :], in_=pool_in[:, :, :])
                for h in range(2):
                    pend = None
                    for t in range(NTILE):
                        mt = gather_bucket(l, h, t, bi); bi += 1
                        if l < 5 and h == 1 and t == 8:
                            # trigger the next layer's h0 table AllGather
                            # from this layer's gather tail (POST(l,t0..4)
                            # bounces land around now)
                            ag_half((bounceA if l < 4 else bounceB)[0],
                                    (gshA if l < 4 else gshB)[0])
                        pa = agg_bucket(l, h, t, mt)
                        if h == 0:
                            # fold the post-agg bias into the partial copy
                            if l == 1:
                                nc.vector.tensor_copy(
                                    partial[:, t, :fo], pa[:, :fo])
                            else:
                                nc.vector.tensor_tensor(
                                    partial[:, t, :fo], pa[:, :fo],
                                    breps[:, l - 1, :fo] if l < 5
                                    else brep5[:, :],
                                    mybir.AluOpType.add)
                        else:
                            if pend is not None:
                                post_tile.pa = pend[1]
                                post_tile(l, pend[0])
                            pend = (t, pa)
                    if h == 1:
                        post_tile.pa = pend[1]
                        post_tile(l, pend[0])
                        if l < 5:
                            # h1 table AllGather: emitted after POST(l,t9)
                            # exists; hides under AG(h0) + the h0 gather pass
                            ag_half((bounceA if l < 4 else bounceB)[1],
                                    (gshA if l < 4 else gshB)[1])

            # ---- mean pool: AllReduce over cores ----
            pres = wp.tile([64, DOUT], dt.float32, tag="pres")
            nc.vector.tensor_copy(pres[:, :], pp[:, :])
            nc.sync.dma_start(out=pool_bounce[:, :], in_=pres[:, :])
            nc.gpsimd.collective_compute(
                "AllReduce", mybir.AluOpType.add,
                replica_groups=[list(range(C))],
                ins=[pool_bounce.ap().opt()],
                outs=[pool_sh.ap().opt()])
            ores = wp.tile([64, DOUT], dt.float32, tag="ores")
            nc.sync.dma_start(out=ores[:, :], in_=pool_sh[:, :])
            nc.sync.dma_start(out=out[:, :], in_=ores[:, :])

    nc.compile()
    return nc


_CACHE = {}


def _get_program(meta):
    if meta not in _CACHE:
        _CACHE[meta] = _build(meta)
    return _CACHE[meta]


def make_in_maps(inputs):
    edge_index = np.asarray(inputs["edge_index"])
    batch = np.asarray(inputs["batch"])
    x = np.asarray(inputs["x"], dtype=np.float32)
    gidx_w, seg, pool, meta = _preprocess(edge_index, batch)
    xt0, xt1 = _xtables(x)
    ident = np.eye(128, dtype=np.float16)
    in_maps = []
    for c in range(C):
        m = {
            "xt0_in": xt0,
            "xt1_in": xt1,
            "seg_in": np.ascontiguousarray(seg[c]),
            "gidx_in": gidx_w[c],
            "pool_in": np.ascontiguousarray(pool[c]),
            "id_in": ident,
        }
        for i in range(5):
            w = np.asarray(inputs[f"W{i + 1}"], dtype=np.float32)
            b = np.asarray(inputs[f"b{i + 1}"], dtype=np.float32)
            m[f"w{i}_in"] = w
            m[f"b{i}_in"] = np.ascontiguousarray(np.tile(b[None, :], (128, 1)))
        in_maps.append(m)
    return in_maps, meta


def kernel(**inputs):
    in_maps, meta = make_in_maps(inputs)
    nc = _get_program(meta)
    res = bass_utils.run_bass_kernel_spmd(
        nc, in_maps, core_ids=list(range(C)))
    return res.results[0]["out"].astype(np.float32)
